# revision 47
# baseline (speedup 1.0000x reference)
"""Multi-head causal self-attention (B=1, S=4096, D=1024, H=16) on 8 TRN2 cores.

Sharding: 2 heads per core (head/tensor parallel). Each core computes its
heads' Q/K/V projections, causal flash attention, and a partial output
projection against its 128 columns of Wo. The host sums the 8 partials and
adds the output bias.

Device layouts (per core, bf16 compute):
  - x is fed transposed:  xT [D=1024, S=4096]   (model dim on partitions)
  - Q^T, K^T [128, 4096]: per-core head dims on partitions (h0: 0-63, h1: 64-127)
  - V natural [4096, 130]: per seq-tile [128, 65*2] = [V_h0 | ones | V_h1 | ones]
    The ones column makes the PV matmul also produce the softmax denominator.
  - scores are computed transposed S^T[k, q] so the PV matmul needs no
    transposition; softmax is exp-only (scores are bounded, no max-subtract).
  - output is written transposed outT [1024, 4096] fp32 (partial; host sums).
"""

import numpy as np
import ml_dtypes
from contextlib import ExitStack

import concourse.bass as bass
import concourse.tile as tile
from concourse import bacc, mybir
from concourse.bass_utils import run_bass_kernel_spmd

P = 128
S = 4096
D = 1024
DH = 64
N_CORES = 8
SCALE = 1.0 / 8.0  # 1/sqrt(64)
NQ = 512           # query block (matmul free dim)
KT = 128           # key tile (contraction partitions)
NQB = S // NQ      # 8 query blocks
NKT = S // KT      # 32 key tiles
KO = D // P        # 8 contraction subtiles over the model dim

BF16 = mybir.dt.bfloat16
F32 = mybir.dt.float32
EXP = mybir.ActivationFunctionType.Exp
ADD = mybir.AluOpType.add


def _emit(tc, xT, wqT, wkT, wvT, woT, bqk, masks, outT, dbg=None):
    nc = tc.nc
    with ExitStack() as ctx:
        from collections import deque
        from concourse.masks import make_identity

        const = ctx.enter_context(tc.tile_pool(name="const", bufs=1))

        # weights/bias/masks first so block-0 projections can start as soon as
        # x chunk 0 lands. All inputs are host-prepacked per-partition
        # contiguous so each DMA is 128 large descriptors, not thousands of
        # 256B ones (those took ~20us to drain at kernel start).
        # bqk's tiny per-partition descriptors must hit the DMA engines before
        # the 8MB of x traffic or its completion strands the first bias-add;
        # the rest spreads across engine queues so x chunk 0 and the weights
        # land in parallel
        bqk_sb = const.tile([P, 3], F32)
        nc.sync.dma_start(bqk_sb, bqk)
        xT_sb = const.tile([P, NQB, KO, NQ], BF16)
        for ko in range(0, KO, 2):  # split x0 so the first q-proj item starts early
            nc.sync.dma_start(xT_sb[:, 0, ko:ko + 2], xT[:, 0, ko:ko + 2])
        wq_sb = const.tile([P, KO, P], BF16)
        nc.scalar.dma_start(wq_sb, wqT)
        wk_sb = const.tile([P, KO, P], BF16)
        nc.scalar.dma_start(wk_sb, wkT)
        wv_sb = const.tile([P, KO, P], BF16)
        nc.gpsimd.dma_start(wv_sb, wvT)
        masks_sb = const.tile([P, 4, NQ], BF16)
        nc.gpsimd.dma_start(masks_sb, masks)
        wo_sb = const.tile([P, D], BF16)
        nc.gpsimd.dma_start(wo_sb, woT)
        for n in range(1, NQB):
            nc.sync.dma_start(xT_sb[:, n], xT[:, n])

        qT_sb = const.tile([P, S], BF16)
        kT_sb = const.tile([P, S], BF16)
        vT_sb = const.tile([P, S], BF16)
        v_sb = const.tile([P, S // P, 130], BF16)
        attnT_sb = const.tile([P, S], BF16)
        nc.vector.memset(v_sb, 1.0)  # presets the two ones-columns

        ident = const.tile([P, P], BF16)
        make_identity(nc, ident)
        ones_bf = const.tile([1, DH], BF16)
        nc.vector.memset(ones_bf, 1.0)  # K=1 broadcast lhsT (bf16: fp32 MMs 5x slower)

        # Warm the PE clock (HAM) with throwaway matmuls while the input DMAs
        # land (the projections themselves finish the warmup).
        with tc.tile_pool(name="warm_psum", bufs=1, space="PSUM") as wpool:
            wt = wpool.tile([P, P], F32)
            for _ in range(34):
                nc.tensor.matmul(wt, lhsT=ident, rhs=ident, start=True, stop=True)

        # PSUM budget (8 banks): spool 4 (two [128,2,512] score slabs),
        # vpool 2 (pv0/pv1 accumulators), ppool 1 (proj accum / V transpose),
        # opool 1 (output projection).
        spool = ctx.enter_context(tc.tile_pool(name="score_psum", bufs=2, space="PSUM"))
        vpool = ctx.enter_context(tc.tile_pool(name="pv_psum", bufs=1, space="PSUM"))
        ppool = ctx.enter_context(tc.tile_pool(name="proj_psum", bufs=1, space="PSUM"))
        opool = ctx.enter_context(tc.tile_pool(name="oproj_psum", bufs=1, space="PSUM"))
        work = ctx.enter_context(tc.tile_pool(name="work", bufs=5))
        nwork = ctx.enter_context(tc.tile_pool(name="nwork", bufs=3))
        dpool = ctx.enter_context(tc.tile_pool(name="dscratch", bufs=2, space="DRAM"))

        def proj_chunk(bcol, w_sb, dst, n, pool_sel=None):
            """Four pacing items of 2 accumulation matmuls each (shared psum).
            Fine granularity keeps the PE FIFO from starving the exp conveyor."""
            state = {}
            pool, tagn = pool_sel or (ppool, "ps")

            def item(j):
                def emit():
                    if j == 0:
                        ps = pool.tile([P, NQ], F32, tag=tagn, name=f"ps_{bcol}_{n}")
                        state["ps"] = ps
                    ps = state["ps"]
                    for kt in range(2 * j, 2 * j + 2):
                        nc.tensor.matmul(
                            ps,
                            lhsT=w_sb[:, kt, :],
                            rhs=xT_sb[:, n, kt, :],
                            start=(kt == 0),
                            stop=(kt == KO - 1),
                        )
                    if j == KO // 2 - 1:
                        nc.vector.tensor_tensor(
                            dst[:, n * NQ:(n + 1) * NQ],
                            ps,
                            bqk_sb[:, bcol:bcol + 1].to_broadcast([P, NQ]),
                            op=ADD,
                        )
                return emit

            return [item(j) for j in range(KO // 2)]

        def v_transpose(t, pool_sel=None):
            pool, tagn = pool_sel or (ppool, "ps")

            def emit():
                tp = pool.tile([P, P], BF16, tag=tagn, name=f"tp_{t}")
                nc.tensor.transpose(tp, vT_sb[:, t * P:(t + 1) * P], ident)
                nc.vector.tensor_copy(
                    v_sb[:, t, :].rearrange("p (h x) -> p h x", x=65)[:, :, 0:DH],
                    tp.rearrange("p (h x) -> p h x", x=DH),
                )
            return emit

        # chunks ping-pong between the proj and oproj psum banks so each
        # stage's matmuls overlap the previous stage's DVE read
        PP, OP = None, None  # bound after the pools exist (below)

        def q_items(nb):
            # deadline: emitted before block nb's first score
            return proj_chunk(0, wq_sb, qT_sb, nb, pool_sel=PP)

        def kv_items(nb):
            # deadline: emitted before block nb's diagonal key tiles (kt=4nb)
            ops = []
            ops += proj_chunk(1, wk_sb, kT_sb, nb, pool_sel=OP)
            ops += proj_chunk(2, wv_sb, vT_sb, nb, pool_sel=PP)
            ops += [v_transpose(t, pool_sel=(OP if t % 2 == 0 else PP))
                    for t in range(4 * nb, 4 * nb + 4)]
            return ops

        def oproj_mtile(b, m, alt=False):
            def emit():
                qsl = slice(b * NQ, (b + 1) * NQ)
                if alt:  # tail: rotate over 4 psum banks (proj/pv rings are idle)
                    pool, tagn = [(opool, "po"), (ppool, "ps"),
                                  (vpool, "pv0"), (vpool, "pv1")][m % 4]
                else:
                    pool, tagn = opool, "po"
                po = pool.tile([P, NQ], F32, tag=tagn, name=f"po_{b}_{m}")
                nc.tensor.matmul(
                    po,
                    lhsT=wo_sb[:, m * P:(m + 1) * P],
                    rhs=attnT_sb[:, qsl],
                    start=True,
                    stop=True,
                )
                ot = work.tile([P, NQ], BF16, tag="ot", name=f"ot_{b}_{m}")
                if alt and m % 2 == 1:
                    nc.scalar.copy(ot, po)
                else:
                    nc.vector.tensor_copy(ot, po)
                dma_eng = nc.sync if m % 2 == 1 else nc.gpsimd
                dma_eng.dma_start(
                    outT.rearrange("(mo p) n -> p mo n", p=P)[:, m, qsl], ot
                )
            return emit

        PP, OP = (ppool, "ps"), (opool, "po")
        # block 0's projections up front
        for op in q_items(0) + kv_items(0):
            op()

        # global pacing queues, consumed one item per kt slot; proj items have
        # emission deadlines (q: block start, kv: block's diagonal) enforced by
        # the force-drains below; oproj has no deadline and drains in slack
        pend_q, pend_kv = {}, {}
        oproj_q = deque()
        for b in range(NQB):
            nk = 4 * (b + 1)  # causal: only key tiles up to the diagonal
            if b + 1 < NQB:
                pend_q[b + 1] = deque(q_items(b + 1))
                pend_kv[b + 1] = deque(kv_items(b + 1))
            for j in [j for j in pend_q if j <= b]:
                while pend_q[j]:
                    pend_q[j].popleft()()
                del pend_q[j]
            for j in [j for j in pend_kv if j < b]:
                while pend_kv[j]:
                    pend_kv[j].popleft()()
                del pend_kv[j]
            if b > 0:
                oproj_q.extend(oproj_mtile(b - 1, m) for m in range(D // P))
            pvs = [
                vpool.tile([DH + 1, NQ], F32, tag=f"pv{h}", name=f"pv{h}_{b}")
                for h in (0, 1)
            ]

            def emit_pv(st):
                pT, kt, q0, nq = st
                for h in (0, 1):
                    nc.tensor.matmul(
                        pvs[h][:, q0:],
                        lhsT=v_sb[:, kt, h * 65:(h + 1) * 65],
                        rhs=pT[:, h, :nq],
                        start=(kt == 0),
                        stop=(kt == nk - 1),
                    )

            def pop_item(kt):
                # deadline order: this block's k/v (due at kt=4b) outranks the
                # next block's q (due at the next block boundary)
                if b in pend_kv and pend_kv[b]:
                    pend_kv[b].popleft()()
                    return
                for pend in (pend_q, pend_kv):
                    for jj in sorted(pend):
                        if pend[jj]:
                            pend[jj].popleft()()
                            return
                if oproj_q and kt >= 4:
                    oproj_q.popleft()()
                    # drain double only under backlog pressure; two pops
                    # per slot exceeds the PE slack behind the exp conveyor
                    if len(oproj_q) > 10:
                        oproj_q.popleft()()

            prev = None  # PV runs one k-tile behind the scores/exp pipeline
            for kt in range(nk):
                j = kt - 4 * b  # >= 0 on causal-diagonal key tiles
                if kt == 4 * b and b in pend_kv:  # diagonal needs this block's k/v
                    while pend_kv[b]:
                        pend_kv[b].popleft()()
                    del pend_kv[b]
                # on diagonal tiles only queries >= 128j can attend this tile
                q0 = max(0, j) * KT
                nq = NQ - q0
                qs0 = b * NQ + q0
                slab = spool.tile([P, 2, NQ], F32, tag="slab")
                for h in (0, 1):
                    nc.tensor.matmul(
                        slab[:, h, :nq],
                        lhsT=kT_sb[h * DH:(h + 1) * DH, kt * KT:(kt + 1) * KT],
                        rhs=qT_sb[h * DH:(h + 1) * DH, qs0:qs0 + nq],
                        start=True,
                        stop=True,
                    )
                pT = work.tile([P, 2, NQ], BF16, tag="pT")
                nc.scalar.activation(pT[:, :, :nq], slab[:, :, :nq], EXP, scale=SCALE)
                if j >= 0:
                    for h in (0, 1):
                        nc.vector.tensor_mul(
                            pT[:, h, :nq],
                            pT[:, h, :nq],
                            masks_sb[:, j, q0:],
                        )
                if prev is not None:
                    emit_pv(prev)
                prev = (pT, kt, q0, nq)
                pop_item(kt)
            emit_pv(prev)
            qsl = slice(b * NQ, (b + 1) * NQ)
            if b < NQB - 1:
                # normalize via DRAM-bounce denominator broadcast; the round-trip
                # latency hides under the next block's flash loop. Both PSUM
                # copies go first: the next block's first PV reuses these slots
                pvSs = []
                for h in (0, 1):
                    pvS = nwork.tile([DH + 1, NQ], F32, tag="pvS")
                    nc.vector.tensor_copy(pvS, pvs[h])  # frees the PSUM slot
                    pvSs.append(pvS)
                for h in (0, 1):
                    pvS = pvSs[h]
                    # recip_approx only works at base partition 0 on HW: copy
                    # the denominator row down first, invert in place there
                    rcp0 = nwork.tile([1, NQ], F32, tag="rcp0")
                    nc.vector.tensor_copy(rcp0, pvS[DH:DH + 1, :])
                    nc.vector.reciprocal_approx_fast(rcp0, rcp0)
                    scr = dpool.tile([NQ], F32, tag="scr")
                    nc.sync.dma_start(scr, rcp0)
                    rb = nwork.tile([DH, NQ], F32, tag="rb")
                    nc.sync.dma_start(rb, scr[None, :].to_broadcast([DH, NQ]))
                    tmp = nwork.tile([DH, NQ], BF16, tag="tmp")
                    nc.vector.tensor_mul(tmp, pvS[0:DH, :], rb)
                    nc.sync.dma_start(attnT_sb[h * DH:(h + 1) * DH, qsl], tmp)
                    if dbg is not None and b == 1:
                        nc.sync.dma_start(dbg[f"pv{h}"], pvS)
            else:
                # tail: no next block to hide DMA latency under — broadcast the
                # denominator with a K=1 matmul instead (PE is idle here)
                pvSs, rcbs = {}, {}
                for h in (1, 0):  # h1 first: its attnT needs a partition-shift DMA
                    pvS = nwork.tile([DH + 1, NQ], F32, tag="pvS")
                    nc.vector.tensor_copy(pvS, pvs[h])
                    rcp0 = nwork.tile([1, NQ], F32, tag="rcp0")
                    nc.vector.tensor_copy(rcp0, pvS[DH:DH + 1, :])
                    nc.vector.reciprocal_approx_fast(rcp0, rcp0)
                    rcb = nwork.tile([1, NQ], BF16, tag="rcb")
                    nc.scalar.copy(rcb, rcp0)
                    pvSs[h], rcbs[h] = pvS, rcb
                for h in (1, 0):
                    pool, tagn = (opool, "po") if h == 1 else (ppool, "ps")
                    dbc = pool.tile([P, NQ], F32, tag=tagn, name=f"dbc_{h}")
                    nc.tensor.matmul(
                        dbc[0:DH, :],
                        lhsT=ones_bf,
                        rhs=rcbs[h],
                        start=True,
                        stop=True,
                    )
                    if h == 0:  # partitions already line up: write attnT directly
                        nc.vector.tensor_mul(
                            attnT_sb[0:DH, qsl], pvSs[h][0:DH, :], dbc[0:DH, :]
                        )
                    else:
                        tmp = nwork.tile([DH, NQ], BF16, tag="tmp")
                        nc.vector.tensor_mul(tmp, pvSs[h][0:DH, :], dbc[0:DH, :])
                        nc.sync.dma_start(attnT_sb[DH:2 * DH, qsl], tmp)
        for m in range(D // P):
            oproj_q.append(oproj_mtile(NQB - 1, m, alt=True))
        while oproj_q:
            oproj_q.popleft()()
        if dbg is not None:
            nc.sync.dma_start(dbg["qT"], qT_sb)
            nc.sync.dma_start(dbg["kT"], kT_sb)
            nc.sync.dma_start(dbg["v"], v_sb)
            nc.sync.dma_start(dbg["attnT"], attnT_sb)


def build(debug_out=False):
    nc = bacc.Bacc(
        "TRN2",
        target_bir_lowering=False,
        debug=False,
        enable_asserts=False,
    )
    xT = nc.dram_tensor("xT", [P, NQB, KO, NQ], BF16, kind="ExternalInput").ap()
    wqT = nc.dram_tensor("wqT", [P, KO, P], BF16, kind="ExternalInput").ap()
    wkT = nc.dram_tensor("wkT", [P, KO, P], BF16, kind="ExternalInput").ap()
    wvT = nc.dram_tensor("wvT", [P, KO, P], BF16, kind="ExternalInput").ap()
    woT = nc.dram_tensor("woT", [P, D], BF16, kind="ExternalInput").ap()
    bqk = nc.dram_tensor("bqk", [P, 3], F32, kind="ExternalInput").ap()
    masks = nc.dram_tensor("masks", [P, 4, NQ], BF16, kind="ExternalInput").ap()
    outT = nc.dram_tensor("outT", [D, S], BF16, kind="ExternalOutput").ap()
    dbg = None
    if debug_out:
        dbg = {
            "qT": nc.dram_tensor("dbg_qT", [P, S], BF16, kind="ExternalOutput").ap(),
            "kT": nc.dram_tensor("dbg_kT", [P, S], BF16, kind="ExternalOutput").ap(),
            "v": nc.dram_tensor("dbg_v", [P, S // P, 130], BF16, kind="ExternalOutput").ap(),
            "attnT": nc.dram_tensor("dbg_attnT", [P, S], BF16, kind="ExternalOutput").ap(),
            "pv0": nc.dram_tensor("dbg_pv0", [DH + 1, NQ], F32, kind="ExternalOutput").ap(),
            "pv1": nc.dram_tensor("dbg_pv1", [DH + 1, NQ], F32, kind="ExternalOutput").ap(),
        }

    with tile.TileContext(nc) as tc:
        _emit(tc, xT, wqT, wkT, wvT, woT, bqk, masks, outT, dbg=dbg)
    nc.compile()
    return nc


def _make_masks():
    k = np.arange(P)[:, None]
    q = np.arange(NQ)[None, :]
    m = np.zeros((P, 4, NQ), np.float32)
    for j in range(4):
        m[:, j, :] = ((KT * j + k) <= q).astype(np.float32)
    return m.astype(ml_dtypes.bfloat16)


_STATE = {}


def _prep_inputs(x, Wq, bq, Wk, bk, Wv, bv, Wo, bo):
    """Prepack every input per-partition contiguous so each DMA lowers to 128
    large descriptors (kernel-start latency) instead of thousands of small
    strided reads."""
    bf = ml_dtypes.bfloat16
    # x [S, D] -> [p, n, ko, q] with s = n*NQ+q, d = ko*P+p
    xPre = np.ascontiguousarray(
        np.asarray(x, np.float32).reshape(NQB, NQ, KO, P).transpose(3, 0, 2, 1)
    ).astype(bf)
    masks = _make_masks()
    Wq = np.asarray(Wq, np.float32)
    Wk = np.asarray(Wk, np.float32)
    Wv = np.asarray(Wv, np.float32)
    Wo = np.asarray(Wo, np.float32)
    bq = np.asarray(bq, np.float32)
    bk = np.asarray(bk, np.float32)
    bv = np.asarray(bv, np.float32)

    def wpack(W, r):  # W[r] [m, d] -> [p, ko, m] with d = ko*P+p
        return np.ascontiguousarray(
            W[r].reshape(P, KO, P).transpose(2, 1, 0)
        ).astype(bf)

    in_maps = []
    for c in range(N_CORES):
        r = slice(c * P, (c + 1) * P)
        in_maps.append({
            "xT": xPre,
            "wqT": wpack(Wq, r),
            "wkT": wpack(Wk, r),
            "wvT": wpack(Wv, r),
            "woT": np.ascontiguousarray(Wo[:, r].T).astype(bf),
            "bqk": np.ascontiguousarray(np.stack([bq[r], bk[r], bv[r]], axis=1)),
            "masks": masks,
        })
    return in_maps


def kernel(x, Wq, bq, Wk, bk, Wv, bv, Wo, bo):
    if "nc" not in _STATE:
        _STATE["nc"] = build()
    nc = _STATE["nc"]
    in_maps = _prep_inputs(x, Wq, bq, Wk, bk, Wv, bv, Wo, bo)
    res = run_bass_kernel_spmd(nc, in_maps, core_ids=list(range(N_CORES)))
    total = res.results[0]["outT"].astype(np.float32)
    for c in range(1, N_CORES):
        total = total + res.results[c]["outT"].astype(np.float32)
    out = total.T + np.asarray(bo, np.float32)[None, :]
    return np.ascontiguousarray(out, dtype=np.float32).reshape(1, S, D)



# revision 53
# speedup vs baseline: 1.0190x; 1.0190x over previous
"""Multi-head causal self-attention (B=1, S=4096, D=1024, H=16) on 8 TRN2 cores.

Sharding: 2 heads per core (head/tensor parallel). Each core computes its
heads' Q/K/V projections, causal flash attention, and a partial output
projection against its 128 columns of Wo. The host sums the 8 partials and
adds the output bias.

Device layouts (per core, bf16 compute):
  - x is fed transposed:  xT [D=1024, S=4096]   (model dim on partitions)
  - Q^T, K^T [128, 4096]: per-core head dims on partitions (h0: 0-63, h1: 64-127)
  - V natural [4096, 130]: per seq-tile [128, 65*2] = [V_h0 | ones | V_h1 | ones]
    The ones column makes the PV matmul also produce the softmax denominator.
  - scores are computed transposed S^T[k, q] so the PV matmul needs no
    transposition; softmax is exp-only (scores are bounded, no max-subtract).
  - output is written transposed outT [1024, 4096] bf16 (partial; host sums
    the 8 cores' partials in fp32 and adds the output bias).

Schedule: the exp conveyor on the Scalar engine (one [128, 2, nq] ACTIVATE
per key tile, ~1.15us each) is the steady-state bottleneck; all projection /
output-projection work is emitted in <=2-matmul pacing items into the PE FIFO
so scores are never starved. Inputs are host-prepacked per-partition
contiguous so the initial DMAs are 128 large descriptors each.
"""

import numpy as np
import ml_dtypes
from contextlib import ExitStack

import concourse.bass as bass
import concourse.tile as tile
from concourse import bacc, mybir
from concourse.bass_utils import run_bass_kernel_spmd

P = 128
S = 4096
D = 1024
DH = 64
N_CORES = 8
SCALE = 1.0 / 8.0  # 1/sqrt(64)
NQ = 512           # query block (matmul free dim)
KT = 128           # key tile (contraction partitions)
NQB = S // NQ      # 8 query blocks
NKT = S // KT      # 32 key tiles
KO = D // P        # 8 contraction subtiles over the model dim

BF16 = mybir.dt.bfloat16
F32 = mybir.dt.float32
EXP = mybir.ActivationFunctionType.Exp
ADD = mybir.AluOpType.add


def _emit(tc, xT, wqT, wkT, wvT, woT, bqk, masks, outT, dbg=None):
    nc = tc.nc
    with ExitStack() as ctx:
        from collections import deque
        from concourse.masks import make_identity

        const = ctx.enter_context(tc.tile_pool(name="const", bufs=1))

        # weights/bias/masks first so block-0 projections can start as soon as
        # x chunk 0 lands. All inputs are host-prepacked per-partition
        # contiguous so each DMA is 128 large descriptors, not thousands of
        # 256B ones (those took ~20us to drain at kernel start).
        # bqk's tiny per-partition descriptors must hit the DMA engines before
        # the 8MB of x traffic or its completion strands the first bias-add;
        # the rest spreads across engine queues so x chunk 0 and the weights
        # land in parallel
        bqk_sb = const.tile([P, 3], F32)
        nc.sync.dma_start(bqk_sb, bqk)
        xT_sb = const.tile([P, NQB, KO, NQ], BF16)
        for ko in range(0, KO, 2):  # split x0 so the first q-proj item starts early
            nc.sync.dma_start(xT_sb[:, 0, ko:ko + 2], xT[:, 0, ko:ko + 2])
        wq_sb = const.tile([P, KO, P], BF16)
        nc.scalar.dma_start(wq_sb, wqT)
        wk_sb = const.tile([P, KO, P], BF16)
        nc.scalar.dma_start(wk_sb, wkT)
        wv_sb = const.tile([P, KO, P], BF16)
        nc.gpsimd.dma_start(wv_sb, wvT)
        masks_sb = const.tile([P, 4, NQ], BF16)
        nc.gpsimd.dma_start(masks_sb, masks)
        wo_sb = const.tile([P, D], BF16)
        nc.gpsimd.dma_start(wo_sb, woT)
        for n in range(1, NQB):
            nc.sync.dma_start(xT_sb[:, n], xT[:, n])

        qT_sb = const.tile([P, S], BF16)
        kT_sb = const.tile([P, S], BF16)
        vT_sb = const.tile([P, S], BF16)
        v_sb = const.tile([P, S // P, 130], BF16)
        attnT_sb = const.tile([P, S], BF16)
        nc.vector.memset(v_sb, 1.0)  # presets the two ones-columns

        ident = const.tile([P, P], BF16)
        make_identity(nc, ident)
        ones_bf = const.tile([1, DH], BF16)
        nc.vector.memset(ones_bf, 1.0)  # K=1 broadcast lhsT (bf16: fp32 MMs 5x slower)

        # Warm the PE clock (HAM) with throwaway matmuls while the input DMAs
        # land (the projections themselves finish the warmup).
        with tc.tile_pool(name="warm_psum", bufs=1, space="PSUM") as wpool:
            wt = wpool.tile([P, P], F32)
            for _ in range(34):
                nc.tensor.matmul(wt, lhsT=ident, rhs=ident, start=True, stop=True)

        # PSUM budget (8 banks): spool 4 (two [128,2,512] score slabs),
        # vpool 2 (pv0/pv1 accumulators), ppool 1 (proj accum / V transpose),
        # opool 1 (output projection).
        spool = ctx.enter_context(tc.tile_pool(name="score_psum", bufs=2, space="PSUM"))
        vpool = ctx.enter_context(tc.tile_pool(name="pv_psum", bufs=1, space="PSUM"))
        ppool = ctx.enter_context(tc.tile_pool(name="proj_psum", bufs=1, space="PSUM"))
        opool = ctx.enter_context(tc.tile_pool(name="oproj_psum", bufs=1, space="PSUM"))
        work = ctx.enter_context(tc.tile_pool(name="work", bufs=5))
        nwork = ctx.enter_context(tc.tile_pool(name="nwork", bufs=3))
        dpool = ctx.enter_context(tc.tile_pool(name="dscratch", bufs=2, space="DRAM"))

        def proj_chunk(bcol, w_sb, dst, n, pool_sel=None):
            """Four pacing items of 2 accumulation matmuls each (shared psum).
            Fine granularity keeps the PE FIFO from starving the exp conveyor."""
            state = {}
            pool, tagn = pool_sel or (ppool, "ps")

            def item(j):
                def emit():
                    if j == 0:
                        ps = pool.tile([P, NQ], F32, tag=tagn, name=f"ps_{bcol}_{n}")
                        state["ps"] = ps
                    ps = state["ps"]
                    for kt in range(2 * j, 2 * j + 2):
                        nc.tensor.matmul(
                            ps,
                            lhsT=w_sb[:, kt, :],
                            rhs=xT_sb[:, n, kt, :],
                            start=(kt == 0),
                            stop=(kt == KO - 1),
                        )
                    if j == KO // 2 - 1:
                        nc.vector.tensor_tensor(
                            dst[:, n * NQ:(n + 1) * NQ],
                            ps,
                            bqk_sb[:, bcol:bcol + 1].to_broadcast([P, NQ]),
                            op=ADD,
                        )
                return emit

            return [item(j) for j in range(KO // 2)]

        def v_transpose(t, pool_sel=None):
            pool, tagn = pool_sel or (ppool, "ps")

            def emit():
                tp = pool.tile([P, P], BF16, tag=tagn, name=f"tp_{t}")
                nc.tensor.transpose(tp, vT_sb[:, t * P:(t + 1) * P], ident)
                nc.vector.tensor_copy(
                    v_sb[:, t, :].rearrange("p (h x) -> p h x", x=65)[:, :, 0:DH],
                    tp.rearrange("p (h x) -> p h x", x=DH),
                )
            return emit

        # chunks ping-pong between the proj and oproj psum banks so each
        # stage's matmuls overlap the previous stage's DVE read
        PP, OP = None, None  # bound after the pools exist (below)

        def q_items(nb):
            # deadline: emitted before block nb's first score
            return proj_chunk(0, wq_sb, qT_sb, nb, pool_sel=PP)

        def kv_items(nb):
            # deadline: emitted before block nb's diagonal key tiles (kt=4nb)
            ops = []
            ops += proj_chunk(1, wk_sb, kT_sb, nb, pool_sel=OP)
            ops += proj_chunk(2, wv_sb, vT_sb, nb, pool_sel=PP)
            ops += [v_transpose(t, pool_sel=(OP if t % 2 == 0 else PP))
                    for t in range(4 * nb, 4 * nb + 4)]
            return ops

        def oproj_mtile(b, m, alt=False):
            def emit():
                qsl = slice(b * NQ, (b + 1) * NQ)
                if alt:  # tail: rotate over 4 psum banks (proj/pv rings are idle)
                    pool, tagn = [(opool, "po"), (ppool, "ps"),
                                  (vpool, "pv0"), (vpool, "pv1")][m % 4]
                else:
                    pool, tagn = opool, "po"
                po = pool.tile([P, NQ], F32, tag=tagn, name=f"po_{b}_{m}")
                nc.tensor.matmul(
                    po,
                    lhsT=wo_sb[:, m * P:(m + 1) * P],
                    rhs=attnT_sb[:, qsl],
                    start=True,
                    stop=True,
                )
                ot = work.tile([P, NQ], BF16, tag="ot", name=f"ot_{b}_{m}")
                if alt and m % 2 == 1:
                    nc.scalar.copy(ot, po)
                else:
                    nc.vector.tensor_copy(ot, po)
                dma_eng = nc.sync if m % 2 == 1 else nc.gpsimd
                dma_eng.dma_start(
                    outT.rearrange("(mo p) n -> p mo n", p=P)[:, m, qsl], ot
                )
            return emit

        PP, OP = (ppool, "ps"), (opool, "po")
        # block 0's projections up front
        for op in q_items(0) + kv_items(0):
            op()

        # global pacing queues, consumed one item per kt slot; proj items have
        # emission deadlines (q: block start, kv: block's diagonal) enforced by
        # the force-drains below; oproj has no deadline and drains in slack
        pend_q, pend_kv = {}, {}
        oproj_q = deque()
        for b in range(NQB):
            nk = 4 * (b + 1)  # causal: only key tiles up to the diagonal
            if b + 1 < NQB:
                pend_q[b + 1] = deque(q_items(b + 1))
                pend_kv[b + 1] = deque(kv_items(b + 1))
            for j in [j for j in pend_q if j <= b]:
                while pend_q[j]:
                    pend_q[j].popleft()()
                del pend_q[j]
            for j in [j for j in pend_kv if j < b]:
                while pend_kv[j]:
                    pend_kv[j].popleft()()
                del pend_kv[j]
            if b > 0:
                oproj_q.extend(oproj_mtile(b - 1, m) for m in range(D // P))
            pvs = [
                vpool.tile([DH + 1, NQ], F32, tag=f"pv{h}", name=f"pv{h}_{b}")
                for h in (0, 1)
            ]

            def emit_pv(st):
                pT, kt, q0, nq = st
                for h in (0, 1):
                    nc.tensor.matmul(
                        pvs[h][:, q0:],
                        lhsT=v_sb[:, kt, h * 65:(h + 1) * 65],
                        rhs=pT[:, h, :nq],
                        start=(kt == 0),
                        stop=(kt == nk - 1),
                    )

            def pop_item(kt):
                # block 0's exp conveyor is short diagonal slabs, and block 1
                # stalls at its kt=4 k/v deadline anyway: pre-drain projection
                # items there (free PE slots, exp is idle regardless)
                budget = 3 if b == 0 else (2 if (b == 1 and kt < 4) else 1)
                for _ in range(budget):
                    popped = False
                    for pend in (pend_q, pend_kv):
                        for jj in sorted(pend):
                            if pend[jj]:
                                pend[jj].popleft()()
                                popped = True
                                break
                        if popped:
                            break
                    if not popped:
                        break
                else:
                    return
                if oproj_q and kt >= 4:
                    oproj_q.popleft()()
                    # drain double only under backlog pressure; two pops
                    # per slot exceeds the PE slack behind the exp conveyor
                    if len(oproj_q) > 10:
                        oproj_q.popleft()()

            prev = None  # PV runs one k-tile behind the scores/exp pipeline
            for kt in range(nk):
                j = kt - 4 * b  # >= 0 on causal-diagonal key tiles
                if kt == 4 * b and b in pend_kv:  # diagonal needs this block's k/v
                    while pend_kv[b]:
                        pend_kv[b].popleft()()
                    del pend_kv[b]
                # on diagonal tiles only queries >= 128j can attend this tile
                q0 = max(0, j) * KT
                nq = NQ - q0
                qs0 = b * NQ + q0
                slab = spool.tile([P, 2, NQ], F32, tag="slab")
                for h in (0, 1):
                    nc.tensor.matmul(
                        slab[:, h, :nq],
                        lhsT=kT_sb[h * DH:(h + 1) * DH, kt * KT:(kt + 1) * KT],
                        rhs=qT_sb[h * DH:(h + 1) * DH, qs0:qs0 + nq],
                        start=True,
                        stop=True,
                    )
                pT = work.tile([P, 2, NQ], BF16, tag="pT")
                nc.scalar.activation(pT[:, :, :nq], slab[:, :, :nq], EXP, scale=SCALE)
                if j >= 0:
                    for h in (0, 1):
                        nc.vector.tensor_mul(
                            pT[:, h, :nq],
                            pT[:, h, :nq],
                            masks_sb[:, j, q0:],
                        )
                if prev is not None:
                    emit_pv(prev)
                prev = (pT, kt, q0, nq)
                pop_item(kt)
            emit_pv(prev)
            qsl = slice(b * NQ, (b + 1) * NQ)
            if b < NQB - 1:
                # normalize via DRAM-bounce denominator broadcast; the round-trip
                # latency hides under the next block's flash loop. Both PSUM
                # copies go first: the next block's first PV reuses these slots
                pvSs = []
                for h in (0, 1):
                    pvS = nwork.tile([DH + 1, NQ], F32, tag="pvS")
                    nc.vector.tensor_copy(pvS, pvs[h])  # frees the PSUM slot
                    pvSs.append(pvS)
                for h in (0, 1):
                    pvS = pvSs[h]
                    # recip_approx only works at base partition 0 on HW: copy
                    # the denominator row down first, invert in place there
                    rcp0 = nwork.tile([1, NQ], F32, tag="rcp0")
                    nc.vector.tensor_copy(rcp0, pvS[DH:DH + 1, :])
                    nc.vector.reciprocal_approx_fast(rcp0, rcp0)
                    scr = dpool.tile([NQ], F32, tag="scr")
                    nc.sync.dma_start(scr, rcp0)
                    rb = nwork.tile([DH, NQ], F32, tag="rb")
                    nc.sync.dma_start(rb, scr[None, :].to_broadcast([DH, NQ]))
                    tmp = nwork.tile([DH, NQ], BF16, tag="tmp")
                    nc.vector.tensor_mul(tmp, pvS[0:DH, :], rb)
                    nc.sync.dma_start(attnT_sb[h * DH:(h + 1) * DH, qsl], tmp)
                    if dbg is not None and b == 1:
                        nc.sync.dma_start(dbg[f"pv{h}"], pvS)
            else:
                # tail: no next block to hide DMA latency under — broadcast the
                # denominator with a K=1 matmul instead (PE is idle here)
                pvSs, rcbs = {}, {}
                for h in (1, 0):  # h1 first: its attnT needs a partition-shift DMA
                    pvS = nwork.tile([DH + 1, NQ], F32, tag="pvS")
                    nc.vector.tensor_copy(pvS, pvs[h])
                    rcp0 = nwork.tile([1, NQ], F32, tag="rcp0")
                    nc.vector.tensor_copy(rcp0, pvS[DH:DH + 1, :])
                    nc.vector.reciprocal_approx_fast(rcp0, rcp0)
                    rcb = nwork.tile([1, NQ], BF16, tag="rcb")
                    nc.scalar.copy(rcb, rcp0)
                    pvSs[h], rcbs[h] = pvS, rcb
                for h in (1, 0):
                    pool, tagn = (opool, "po") if h == 1 else (ppool, "ps")
                    dbc = pool.tile([P, NQ], F32, tag=tagn, name=f"dbc_{h}")
                    nc.tensor.matmul(
                        dbc[0:DH, :],
                        lhsT=ones_bf,
                        rhs=rcbs[h],
                        start=True,
                        stop=True,
                    )
                    if h == 0:  # partitions already line up: write attnT directly
                        nc.vector.tensor_mul(
                            attnT_sb[0:DH, qsl], pvSs[h][0:DH, :], dbc[0:DH, :]
                        )
                    else:
                        tmp = nwork.tile([DH, NQ], BF16, tag="tmp")
                        nc.vector.tensor_mul(tmp, pvSs[h][0:DH, :], dbc[0:DH, :])
                        nc.sync.dma_start(attnT_sb[DH:2 * DH, qsl], tmp)
        for m in range(D // P):
            oproj_q.append(oproj_mtile(NQB - 1, m, alt=True))
        while oproj_q:
            oproj_q.popleft()()
        if dbg is not None:
            nc.sync.dma_start(dbg["qT"], qT_sb)
            nc.sync.dma_start(dbg["kT"], kT_sb)
            nc.sync.dma_start(dbg["v"], v_sb)
            nc.sync.dma_start(dbg["attnT"], attnT_sb)


def build(debug_out=False):
    nc = bacc.Bacc(
        "TRN2",
        target_bir_lowering=False,
        debug=False,
        enable_asserts=False,
    )
    xT = nc.dram_tensor("xT", [P, NQB, KO, NQ], BF16, kind="ExternalInput").ap()
    wqT = nc.dram_tensor("wqT", [P, KO, P], BF16, kind="ExternalInput").ap()
    wkT = nc.dram_tensor("wkT", [P, KO, P], BF16, kind="ExternalInput").ap()
    wvT = nc.dram_tensor("wvT", [P, KO, P], BF16, kind="ExternalInput").ap()
    woT = nc.dram_tensor("woT", [P, D], BF16, kind="ExternalInput").ap()
    bqk = nc.dram_tensor("bqk", [P, 3], F32, kind="ExternalInput").ap()
    masks = nc.dram_tensor("masks", [P, 4, NQ], BF16, kind="ExternalInput").ap()
    outT = nc.dram_tensor("outT", [D, S], BF16, kind="ExternalOutput").ap()
    dbg = None
    if debug_out:
        dbg = {
            "qT": nc.dram_tensor("dbg_qT", [P, S], BF16, kind="ExternalOutput").ap(),
            "kT": nc.dram_tensor("dbg_kT", [P, S], BF16, kind="ExternalOutput").ap(),
            "v": nc.dram_tensor("dbg_v", [P, S // P, 130], BF16, kind="ExternalOutput").ap(),
            "attnT": nc.dram_tensor("dbg_attnT", [P, S], BF16, kind="ExternalOutput").ap(),
            "pv0": nc.dram_tensor("dbg_pv0", [DH + 1, NQ], F32, kind="ExternalOutput").ap(),
            "pv1": nc.dram_tensor("dbg_pv1", [DH + 1, NQ], F32, kind="ExternalOutput").ap(),
        }

    with tile.TileContext(nc) as tc:
        _emit(tc, xT, wqT, wkT, wvT, woT, bqk, masks, outT, dbg=dbg)
    nc.compile()
    return nc


def _make_masks():
    k = np.arange(P)[:, None]
    q = np.arange(NQ)[None, :]
    m = np.zeros((P, 4, NQ), np.float32)
    for j in range(4):
        m[:, j, :] = ((KT * j + k) <= q).astype(np.float32)
    return m.astype(ml_dtypes.bfloat16)


_STATE = {}


def _prep_inputs(x, Wq, bq, Wk, bk, Wv, bv, Wo, bo):
    """Prepack every input per-partition contiguous so each DMA lowers to 128
    large descriptors (kernel-start latency) instead of thousands of small
    strided reads."""
    bf = ml_dtypes.bfloat16
    # x [S, D] -> [p, n, ko, q] with s = n*NQ+q, d = ko*P+p
    xPre = np.ascontiguousarray(
        np.asarray(x, np.float32).reshape(NQB, NQ, KO, P).transpose(3, 0, 2, 1)
    ).astype(bf)
    masks = _make_masks()
    Wq = np.asarray(Wq, np.float32)
    Wk = np.asarray(Wk, np.float32)
    Wv = np.asarray(Wv, np.float32)
    Wo = np.asarray(Wo, np.float32)
    bq = np.asarray(bq, np.float32)
    bk = np.asarray(bk, np.float32)
    bv = np.asarray(bv, np.float32)

    def wpack(W, r):  # W[r] [m, d] -> [p, ko, m] with d = ko*P+p
        return np.ascontiguousarray(
            W[r].reshape(P, KO, P).transpose(2, 1, 0)
        ).astype(bf)

    in_maps = []
    for c in range(N_CORES):
        r = slice(c * P, (c + 1) * P)
        in_maps.append({
            "xT": xPre,
            "wqT": wpack(Wq, r),
            "wkT": wpack(Wk, r),
            "wvT": wpack(Wv, r),
            "woT": np.ascontiguousarray(Wo[:, r].T).astype(bf),
            "bqk": np.ascontiguousarray(np.stack([bq[r], bk[r], bv[r]], axis=1)),
            "masks": masks,
        })
    return in_maps


def kernel(x, Wq, bq, Wk, bk, Wv, bv, Wo, bo):
    if "nc" not in _STATE:
        _STATE["nc"] = build()
    nc = _STATE["nc"]
    in_maps = _prep_inputs(x, Wq, bq, Wk, bk, Wv, bv, Wo, bo)
    res = run_bass_kernel_spmd(nc, in_maps, core_ids=list(range(N_CORES)))
    total = res.results[0]["outT"].astype(np.float32)
    for c in range(1, N_CORES):
        total = total + res.results[c]["outT"].astype(np.float32)
    out = total.T + np.asarray(bo, np.float32)[None, :]
    return np.ascontiguousarray(out, dtype=np.float32).reshape(1, S, D)

